# revision 37
# baseline (speedup 1.0000x reference)
"""Multi-head attention block (B=4, N=1024, C=1024, H=16) on 8 TRN2 NeuronCores.

Sharding: hybrid data/tensor parallel. Core c handles batch b = c//2 and head
group g = c%2 (8 of the 16 heads). Each core computes, for its (b, g):
    qkvT = (x_b @ w_qkv[:, cols(g)] + b_qkv[cols(g)])^T      (bf16 matmuls)
    per head: S^T = k q^T ; P^T = exp(S^T/8) ; [outT; den] = [v|1]^T @ P^T
              outT /= den  (PE-broadcast of 1/den + DVE multiply)
    out_partial = outT^T @ w_proj[rows(g), :]
Host sums the two partial outputs per batch (the "all-reduce") + b_proj.

All matmul operands are bfloat16 (fp32 PSUM accumulation). fp32 matmuls on
TRN2 run at 1/4 rate and float32r (~12-bit) at 1/2 rate; bf16 is the only
full-rate dtype, and the measured end-to-end relative error (~6e-3) is well
inside the reference tolerance for this problem family.

Pipelining: units of (head-pair, 512-token q-chunk). Per unit the TensorE
stream interleaves score matmuls of unit u with attention-value matmuls and
normalization of unit u-1, keeping PE busy while ScalarE exponentiates.
"""
from contextlib import ExitStack

import ml_dtypes
import numpy as np

import concourse.bass as bass  # noqa: F401
import concourse.tile as tile
from concourse import bacc, mybir
from concourse.bass_utils import run_bass_kernel_spmd

F32 = mybir.dt.float32
F32R = mybir.dt.float32r
BF16 = mybir.dt.bfloat16

B, N, C, H, D = 4, 1024, 1024, 16, 64
HL = 8          # local heads per core
NPH = HL // 2   # local head pairs
KC = C // 128   # contraction chunks
NT = N // 128   # token blocks
NQ = N // 512   # 512-token chunks
SCALE = D ** -0.5

_CACHE = {}


def _build_nc():
    nc = bacc.Bacc("TRN2", target_bir_lowering=False, debug=False, num_devices=8)

    xt_d = nc.dram_tensor("xt", [128, KC, N], BF16, kind="ExternalInput")
    wqkv_d = nc.dram_tensor("wqkv", [128, NPH, KC, 3 * 128], BF16, kind="ExternalInput")
    bqkv_d = nc.dram_tensor("bqkv", [128, 3 * NPH], F32, kind="ExternalInput")
    wproj_d = nc.dram_tensor("wproj", [128, NPH, C], BF16, kind="ExternalInput")
    ident_d = nc.dram_tensor("ident", [128, 128], BF16, kind="ExternalInput")
    ones64_d = nc.dram_tensor("ones64", [128, 64], BF16, kind="ExternalInput")
    vones_d = nc.dram_tensor("vones", [128, HL, 1], BF16, kind="ExternalInput")
    out_d = nc.dram_tensor("out", [128, NT, C], F32, kind="ExternalOutput")

    with nc.allow_low_precision(reason="f32r compute"), \
         tile.TileContext(nc) as tc, ExitStack() as ctx:
        const = ctx.enter_context(tc.tile_pool(name="const", bufs=1))
        big = ctx.enter_context(tc.tile_pool(name="big", bufs=1))
        wpool = ctx.enter_context(tc.tile_pool(name="wpool", bufs=2))
        qkp_pool = ctx.enter_context(tc.tile_pool(name="qkp", bufs=2))
        vp_pool = ctx.enter_context(tc.tile_pool(name="vp", bufs=1))
        ptp = ctx.enter_context(tc.tile_pool(name="ptp", bufs=6))
        nrm = ctx.enter_context(tc.tile_pool(name="nrm", bufs=3))
        oep = ctx.enter_context(tc.tile_pool(name="oep", bufs=6))
        ps_g = ctx.enter_context(tc.tile_pool(name="ps_g", bufs=3, space="PSUM"))
        ps_sc = ctx.enter_context(tc.tile_pool(name="ps_sc", bufs=2, space="PSUM"))
        ps_av = ctx.enter_context(tc.tile_pool(name="ps_av", bufs=1, space="PSUM"))

        _qs = (nc.gpsimd, nc.scalar, nc.sync)
        ident = const.tile([128, 128], BF16, tag="ident")
        nc.gpsimd.dma_start(ident[:], ident_d.ap())
        ones64 = const.tile([128, 64], BF16, tag="ones64")
        nc.gpsimd.dma_start(ones64[:], ones64_d.ap())
        vones = const.tile([128, HL, 1], BF16, tag="vones")
        nc.gpsimd.dma_start(vones[:], vones_d.ap())
        b_sb = const.tile([128, 3 * NPH], F32, tag="b_sb")
        nc.gpsimd.dma_start(b_sb[:], bqkv_d.ap())

        xt = big.tile([128, KC, N], BF16, tag="xt")
        for kc in range(KC):  # chunked + spread over queues: early start
            # offset by 1 so xt[kc0] (scalar) doesn't queue behind the
            # hoisted w0 chunks (gpsimd/sync)
            _qs[(kc + 1) % 3].dma_start(xt[:, kc, :], xt_d.ap()[:, kc, :])
        v_nat = big.tile([128, NT, HL, D + 1], BF16, tag="v_nat")
        att = big.tile([128, NPH, N], BF16, tag="att")  # normalized outT

        for kb in range(NT):
            nc.vector.tensor_copy(v_nat[:, kb, :, D:D + 1], vones[:])

        # ---------------- continuous software pipeline ----------------
        # Three persistent generators (qkv-qk, qkv-v+transpose, scores/AV)
        # round-robined at ~0.4-0.9us TensorE quanta. Tile resolves data
        # deps, but deps follow EMISSION order, so the marker gates below
        # also guarantee producer-before-consumer emission.
        P = {"w": {}, "qk": {}, "k_full": set(), "qk_full": set(), "pts": {},
             "pending_av": None, "sav_done": -1, "vnat": -1, "wproj": None}

        def emit_E(tbs):
            wproj = P["wproj"]
            for tb in tbs:
                for cc in range(NQ):
                    pp = ps_g.tile([128, 512], F32, tag="g")
                    for fc in range(NPH):
                        nc.tensor.matmul(
                            pp[:], att[:, fc, tb * 128:(tb + 1) * 128],
                            wproj[:, fc, cc * 512:(cc + 1) * 512],
                            start=(fc == 0), stop=(fc == NPH - 1))
                    oe = oep.tile([128, 512], F32, tag="oe")
                    # alternate evictions between DVE and the (by now idle)
                    # ScalarE so neither becomes the projection-stage pacer
                    if (2 * tb + cc) % 2 == 0:
                        nc.vector.tensor_copy(oe[:], pp[:])
                    else:
                        nc.scalar.copy(oe[:], pp[:])
                    eng = (nc.sync, nc.gpsimd, nc.scalar)[(2 * tb + cc) % 3]
                    eng.dma_start(out_d.ap()[:, tb, cc * 512:(cc + 1) * 512], oe[:])

        def qkv_group(w_t, ph, j, qc, dst):
            acc = ps_g.tile([128, 512], F32, tag="g")
            for kc in range(KC):
                nc.tensor.matmul(
                    acc[:], w_t[:, kc, j * 128:(j + 1) * 128],
                    xt[:, kc, qc * 512:(qc + 1) * 512],
                    start=(kc == 0), stop=(kc == KC - 1))
                if kc % 2 == 1 and kc < KC - 1:
                    yield None
            nc.vector.tensor_scalar_add(
                dst, acc[:], b_sb[:, ph * 3 + j:ph * 3 + j + 1])
            yield None

        def gen_Aqk_all():
            for ph in range(NPH):
                while P["sav_done"] < ph - 2:
                    yield None
                if ph in P["w"]:
                    w_t = P["w"][ph]        # pre-hoisted DMA (ph 0)
                else:
                    w_t = wpool.tile([128, KC, 3 * 128], BF16, tag="w")
                    nc.sync.dma_start(w_t[:], wqkv_d.ap()[:, ph, :, :])
                    P["w"][ph] = w_t
                qk = qkp_pool.tile([128, 2, N], BF16, tag="qk")
                # Emission gates: scores kbp0-1 need q0+k0 emitted, kbp2-3
                # need k1, the qc1 score unit needs q1
                for n_, (j, qc) in enumerate(((0, 0), (1, 0), (1, 1), (0, 1))):
                    yield from qkv_group(w_t, ph, j, qc,
                                         qk[:, j, qc * 512:(qc + 1) * 512])
                    if n_ == 1:
                        P["qk"][ph] = qk
                    elif n_ == 2:
                        P["k_full"].add(ph)
                P["qk_full"].add(ph)

        def gen_Av_all():
            for ph in range(NPH):
                while ph not in P["w"] or P["sav_done"] < ph - 1:
                    yield None
                w_t = P["w"].pop(ph)
                vp = vp_pool.tile([128, N], BF16, tag="vp")
                for qc in range(NQ):
                    yield from qkv_group(w_t, ph, 2, qc,
                                         vp[:, qc * 512:(qc + 1) * 512])
                for tbp in range(NT // 2):
                    for tb in (2 * tbp, 2 * tbp + 1):
                        pst = ps_g.tile([128, 128], BF16, tag="g")
                        nc.tensor.transpose(pst[:], vp[:, tb * 128:(tb + 1) * 128],
                                            ident[:])
                        nc.vector.tensor_copy(
                            v_nat[:, tb, 2 * ph:2 * ph + 2, 0:D],
                            pst[:].rearrange("p (h d) -> p h d", d=D))
                    yield None
                P["vnat"] = ph

        def emit_S_unit(ph, qc, qk):
            """Yields per kb-pair (4 matmuls + 2 batched exps)."""
            pt0 = ptp.tile([128, NT, 512], BF16, tag="pt")
            pt1 = ptp.tile([128, NT, 512], BF16, tag="pt")
            for kbp in range(NT // 2):
                while kbp == 2 and ph not in P["k_full"]:
                    yield None
                pe = ps_sc.tile([128, 2, 512], F32, tag="sc")
                po = ps_sc.tile([128, 2, 512], F32, tag="sc")
                for i, kb in enumerate((2 * kbp, 2 * kbp + 1)):
                    nc.tensor.matmul(
                        pe[:, i, :], qk[0:64, 1, kb * 128:(kb + 1) * 128],
                        qk[0:64, 0, qc * 512:(qc + 1) * 512],
                        start=True, stop=True, tile_position=(0, 0))
                    nc.tensor.matmul(
                        po[:, i, :], qk[64:128, 1, kb * 128:(kb + 1) * 128],
                        qk[64:128, 0, qc * 512:(qc + 1) * 512],
                        start=True, stop=True, tile_position=(64, 0))
                nc.scalar.activation(
                    pt0[:, 2 * kbp:2 * kbp + 2, :], pe[:],
                    mybir.ActivationFunctionType.Exp, scale=SCALE)
                nc.scalar.activation(
                    pt1[:, 2 * kbp:2 * kbp + 2, :], po[:],
                    mybir.ActivationFunctionType.Exp, scale=SCALE)
                yield None
            P["pts"][(ph, qc)] = (pt0, pt1)

        def emit_AV_unit(ph, qc):
            """Fine-grained AV + normalize for both heads of (ph, qc)."""
            while P["vnat"] < ph:      # v_nat writes must be emitted first
                yield None
            pt0, pt1 = P["pts"].pop((ph, qc))
            sts, rhs_ = [], []
            for hi, pt in ((0, pt0), (1, pt1)):
                h = 2 * ph + hi
                avp = ps_av.tile([D + 1, 512], F32, tag="av")
                for kb in range(NT):
                    nc.tensor.matmul(
                        avp[:], v_nat[:, kb, h, :],
                        pt[:, kb, :], start=(kb == 0), stop=(kb == NT - 1))
                    if kb % 2 == 1 and kb < NT - 1:
                        yield None
                st = nrm.tile([D + 1, 512], F32, tag="st65")
                nc.vector.tensor_copy(st[:], avp[:])
                sts.append(st)
                # start this head's reciprocal chain immediately (DVE/DMA
                # runs under the other head's PE work); custom-DVE ops can't
                # shift partitions, so DMA the denom row to partition 0
                dent = nrm.tile([1, 512], F32, tag="dent")
                nc.sync.dma_start(dent[:], st[D:D + 1, :])
                rhf = nrm.tile([1, 512], F32, tag="rhf")
                nc.vector.reciprocal_approx_fast(rhf[:], dent[:])
                rh = nrm.tile([1, 512], BF16, tag="rh")
                nc.vector.tensor_copy(rh[:], rhf[:])
                rhs_.append(rh)
                yield None
            for hi in range(2):
                st, rh = sts[hi], rhs_[hi]
                bc = ps_av.tile([64, 512], F32, tag="av")
                nc.tensor.matmul(bc[:], ones64[0:1, :], rh[:],
                                 start=True, stop=True)
                stn = nrm.tile([D, 512], BF16, tag="stn")
                nc.vector.tensor_mul(stn[:], st[0:D, :], bc[:])
                nc.sync.dma_start(
                    att[hi * D:(hi + 1) * D, ph, qc * 512:(qc + 1) * 512],
                    stn[:])
                yield None

        def gen_SAV_all():
            for ph in range(NPH):
                while ph not in P["qk"]:
                    av = P["pending_av"]
                    if av is not None:
                        next(av, None)
                    yield None
                qk = P["qk"].pop(ph)
                if ph == NPH - 2:
                    # prefetch wproj into a freed PT slot
                    wproj_t = ptp.tile([128, NPH, C], BF16, tag="pt")
                    P["wproj"] = wproj_t
                    for fc in range(NPH):
                        nc.gpsimd.dma_start(wproj_t[:, fc, :],
                                            wproj_d.ap()[:, fc, :])
                for qc in range(NQ):
                    while qc == 1 and ph not in P["qk_full"]:
                        av = P["pending_av"]
                        if av is not None:
                            next(av, None)
                        yield None
                    for _ in emit_S_unit(ph, qc, qk):
                        for _ in range(3):
                            av = P["pending_av"]
                            if av is not None:
                                next(av, None)
                        yield None
                    av = P["pending_av"]
                    if av is not None:
                        # drain, yielding so sibling generators can advance
                        # any markers this AV unit spins on
                        while next(av, _STOP) is not _STOP:
                            yield None
                    if ph == NPH - 1 and qc == 1:
                        # att tokens 0-511 fully normalized: project them
                        # now, while ScalarE works the last exps
                        emit_E(range(NT // 2))
                    P["pending_av"] = emit_AV_unit(ph, qc)
                P["sav_done"] = ph

        # hoist the first weight chunk's DMAs to the very front
        w0 = wpool.tile([128, KC, 3 * 128], BF16, tag="w")
        for kk in range(0, KC, 2):
            (nc.gpsimd if kk < 4 else nc.sync).dma_start(
                w0[:, kk:kk + 2, :], wqkv_d.ap()[:, 0, kk:kk + 2, :])
        P["w"][0] = w0

        _STOP = object()
        gens = [gen_SAV_all(), gen_Aqk_all(), gen_Av_all()]
        while gens:
            for g in list(gens):
                if next(g, _STOP) is _STOP:
                    gens.remove(g)
        av = P["pending_av"]
        if av is not None:
            for _ in av:
                pass

        # ---- output projection (second half; first emitted in gen_SAV) ----
        emit_E(range(NT // 2, NT))

    nc.compile()
    return nc


def _get_nc():
    if _CACHE.get("nc") is None:
        _CACHE["nc"] = _build_nc()
    return _CACHE["nc"]


def _prep_core_inputs(x, w_qkv, b_qkv, g, b):
    cs = 512 * g
    wq = w_qkv[:, 0 * C + cs:0 * C + cs + 512]
    wk = w_qkv[:, 1 * C + cs:1 * C + cs + 512]
    wv = w_qkv[:, 2 * C + cs:2 * C + cs + 512]
    wdev = np.stack([wq.reshape(C, NPH, 128), wk.reshape(C, NPH, 128),
                     wv.reshape(C, NPH, 128)], axis=2)      # [C, ph, 3, 128]
    # -> [128(p), NPH, KC, 384] (contiguous per-partition per-ph blocks)
    wdev = wdev.reshape(KC, 128, NPH, 3 * 128).transpose(1, 2, 0, 3)
    wdev = np.ascontiguousarray(wdev.astype(ml_dtypes.bfloat16))

    bq = b_qkv[0 * C + cs:0 * C + cs + 512]
    bk = b_qkv[1 * C + cs:1 * C + cs + 512]
    bv = b_qkv[2 * C + cs:2 * C + cs + 512]
    bdev = np.stack([bq.reshape(NPH, 128), bk.reshape(NPH, 128),
                     bv.reshape(NPH, 128)], axis=1)          # [ph, 3, 128]
    bdev = np.ascontiguousarray(bdev.reshape(3 * NPH, 128).T)

    xt = np.ascontiguousarray(x[b].T.reshape(KC, 128, N).transpose(1, 0, 2).astype(ml_dtypes.bfloat16))
    return xt, wdev, bdev


def kernel(x, w_qkv, b_qkv, w_proj, b_proj):
    x = np.asarray(x, dtype=np.float32)
    w_qkv = np.asarray(w_qkv, dtype=np.float32)
    b_qkv = np.asarray(b_qkv, dtype=np.float32)
    w_proj = np.asarray(w_proj, dtype=np.float32)
    b_proj = np.asarray(b_proj, dtype=np.float32)

    nc = _get_nc()

    ident = np.eye(128).astype(ml_dtypes.bfloat16)
    ones64 = np.ones((128, 64), dtype=ml_dtypes.bfloat16)
    vones = np.ones((128, HL, 1), dtype=ml_dtypes.bfloat16)

    in_maps = []
    for c in range(8):
        b, g = c // 2, c % 2
        xt, wdev, bdev = _prep_core_inputs(x, w_qkv, b_qkv, g, b)
        wp = w_proj[512 * g:512 * g + 512].reshape(NPH, 2, D, C)
        wp = np.ascontiguousarray(
            wp.transpose(1, 2, 0, 3).reshape(128, NPH, C).astype(ml_dtypes.bfloat16))
        in_maps.append({
            "xt": xt, "wqkv": wdev, "bqkv": bdev, "wproj": wp,
            "ident": ident, "ones64": ones64, "vones": vones,
        })

    res = run_bass_kernel_spmd(nc, in_maps, core_ids=list(range(8)))
    _CACHE["last_results"] = res

    out = np.empty((B, N, C), dtype=np.float32)
    for b in range(B):
        o0 = res.results[2 * b]["out"].transpose(1, 0, 2).reshape(N, C)
        o1 = res.results[2 * b + 1]["out"].transpose(1, 0, 2).reshape(N, C)
        out[b] = o0 + o1 + b_proj[None, :]
    return out
